# revision 1
# baseline (speedup 1.0000x reference)
"""CompoundHeadAttention TRN2 kernel (v2).

Full-input contract: kernel(**inputs) takes the unsharded tensors from
setup_inputs() and returns the full [1, 2048, 2048] float32 output.

Sharding (8 cores, tensor-parallel over the HK=8 kv heads):
  core h owns kv head h: its Wq/Wk/Wv column slice, its WG[h]/bG[h], and
  Wfc row-slice [h*256:(h+1)*256, :].  Each core computes its head's
  attention + its partial FC output [2048, 2048] in fp16; the host sums
  the 8 partials and adds bfc (the "all-reduce" of the row-sharded FC).

v2 device-side design (N=2048, E=2048, D=64, G=4 per core):
  - inputs qT/kT/vT [E, N] fp16 loaded as [128, 4, 1024] "quads", 4 per
    (tensor, window-pair); triggers spread over sync/gpsimd/vector queues
  - projections fp16, M=64 (no partition dup): psum [64, 512] per window
  - G transform fp16 from qt; qg0..3 stored as separate [64, 512] tiles
    at base partition 0 so ST needs no duplicated kt rows
  - ST fp16 [128 s, n] with exact 128-col causal trim (fp16 has no
    small-N matmul penalty); exp on ACT -> pt bf16; diagonal mask via
    DVE multiply with a precomputed triangular bf16 tile
  - PV bf16 accumulated [65, n] (row 64 = softmax denominators via a
    ones column in vo); PV lags ST by one chunk to hide ACT latency
  - normalize: reciprocal_approx_fast (DVE) + partition_broadcast (Pool)
    + tensor_mul -> hid fp16
  - FC fp16: hid [128, 128] stationary x wfc [128, 512]; psum->stage
    fp16 copies alternate DVE/Pool; out DMA per 128-row chunk
"""

import os
import sys

import numpy as np

if "/opt/trn_rl_repo" not in sys.path and os.path.isdir("/opt/trn_rl_repo"):
    sys.path.insert(0, "/opt/trn_rl_repo")

import concourse.bass as bass  # noqa: E402
import concourse.mybir as mybir  # noqa: E402
import concourse.tile as tile  # noqa: E402
from concourse import bacc  # noqa: E402
from concourse import bass_utils  # noqa: E402

F32 = mybir.dt.float32
F16 = mybir.dt.float16
BF16 = mybir.dt.bfloat16
AF = mybir.ActivationFunctionType

N = 2048
E = 2048
HK = 8
D = 64
G = 4
NB = 4         # 512-wide n-windows


def build_program():
    nc = bacc.Bacc("TRN2", target_bir_lowering=False, debug=False,
                   enable_asserts=False)

    # ---- DRAM I/O ----
    # pre-swizzled input layout: [p, qd, P, e, c] = x^T[qd*512+e*128+p,
    # P*1024+c] so one quad = 128 contiguous 8 KB runs (1 descriptor per
    # partition)
    qT = nc.dram_tensor("qT", [128, 4, 2, 4, 1024], F16,
                        kind="ExternalInput").ap()
    kT = nc.dram_tensor("kT", [128, 4, 2, 4, 1024], F16,
                        kind="ExternalInput").ap()
    vT = nc.dram_tensor("vT", [128, 4, 2, 4, 1024], F16,
                        kind="ExternalInput").ap()
    # weight chunk layout: [128, 16*64] — e-chunk ec occupies cols [64ec, 64ec+64)
    wq = nc.dram_tensor("wq", [128, 16 * 128], F16, kind="ExternalInput").ap()
    wk = nc.dram_tensor("wk", [128, 16 * 128], F16, kind="ExternalInput").ap()
    wv = nc.dram_tensor("wv", [128, 16 * 64], F16, kind="ExternalInput").ap()
    bq = nc.dram_tensor("bq", [128, 1], F32, kind="ExternalInput").ap()
    bk = nc.dram_tensor("bk", [128, 1], F32, kind="ExternalInput").ap()
    bv = nc.dram_tensor("bv", [64, 1], F32, kind="ExternalInput").ap()
    wg = nc.dram_tensor("wg", [128, 256], F16, kind="ExternalInput").ap()
    bg = nc.dram_tensor("bg", [128, 2], F32, kind="ExternalInput").ap()
    wfc = nc.dram_tensor("wfc", [256, E], F16, kind="ExternalInput").ap()
    out = nc.dram_tensor("out", [N, E], F16, kind="ExternalOutput").ap()

    dumps = None
    if os.environ.get("KDUMP"):
        dumps = {
            "d_kt0": nc.dram_tensor("d_kt0", [64, 512], F16,
                                    kind="ExternalOutput").ap(),
            "d_qg00": nc.dram_tensor("d_qg00", [64, 512], F16,
                                     kind="ExternalOutput").ap(),
            "d_vo0": nc.dram_tensor("d_vo0", [128, 4, 65], BF16,
                                    kind="ExternalOutput").ap(),
            "d_hid01_0": nc.dram_tensor("d_hid01_0", [128, 512], F16,
                                        kind="ExternalOutput").ap(),
            "d_rec": nc.dram_tensor("d_rec", [64, 512], F32,
                                    kind="ExternalOutput").ap(),
        }

    with tile.TileContext(nc) as tc:
        build_tile_kernel(tc, qT=qT, kT=kT, vT=vT, wq=wq, wk=wk, wv=wv,
                          bq=bq, bk=bk, bv=bv, wg=wg, bg=bg, wfc=wfc,
                          out=out, dumps=dumps)
    nc.compile()
    return nc


def build_tile_kernel(tc, *, qT, kT, vT, wq, wk, wv, bq, bk, bv, wg, bg,
                      wfc, out, dumps=None):
    nc = tc.nc

    import contextlib
    ctx = contextlib.ExitStack()
    ctx.__enter__()
    cp = ctx.enter_context(tc.tile_pool(name="persist", bufs=1))

    def ptile(shape, dtype, name):
        return cp.tile(shape, dtype, tag=name, name=name)

    # ---- persistent constants in SBUF ----
    wq_sb = ptile([128, 16 * 128], F16, "wq_sb")
    wk_sb = ptile([128, 16 * 128], F16, "wk_sb")
    wv_sb = ptile([128, 16 * 64], F16, "wv_sb")
    wg_sb = ptile([128, 256], F16, "wg_sb")
    wfc0_sb = ptile([128, E], F16, "wfc0_sb")
    wfc1_sb = ptile([128, E], F16, "wfc1_sb")
    bq_sb = ptile([128, 1], F32, "bq_sb")
    bk_sb = ptile([128, 1], F32, "bk_sb")
    bv_sb = ptile([64, 1], F32, "bv_sb")
    bg_sb = ptile([128, 2], F32, "bg_sb")
    # causal mask constant: mask[s, n_local] = 1 if n_local >= s else 0,
    # duplicated side by side for the two g-halves of a pair
    mask_sb = ptile([128, 256], BF16, "mask_sb")
    nc.vector.memset(mask_sb[:], 1.0)
    # exp logit shift (cancels in softmax; keeps exp within fp16 range)
    eshift_sb = ptile([128, 1], F32, "eshift_sb")
    nc.vector.memset(eshift_sb[:], -35.0)
    mask3 = mask_sb[:].rearrange("p (h c) -> p h c", c=128)
    nc.gpsimd.affine_select(
        out=mask3, in_=mask3, compare_op=mybir.AluOpType.is_ge,
        fill=0.0, base=0, pattern=[[0, 2], [1, 128]], channel_multiplier=-1)
    # dummy broadcast: preloads the gpsimd pool config for
    # partition_broadcast off the critical path
    warm_sb = ptile([64, 1], F32, "warm_sb")
    nc.gpsimd.partition_broadcast(warm_sb[:], eshift_sb[0:1, 0:1])

    # per-window persistent activations
    kt_w = [ptile([128, 512], F16, f"kt{j}") for j in range(NB)]
    # one tile per 128-token V chunk: DMA transpose writes at offset 0,
    # col 64 holds the ones column (softmax denominator row of PV)
    vo_w = [[ptile([128, 65], BF16, f"vo{j}_{c}") for c in range(4)]
            for j in range(NB)]
    for j in range(NB):
        for c in range(4):
            nc.vector.memset(vo_w[j][c][:, 64:65], 1.0)
    qg01_w = [ptile([128, 512], F16, f"qg01_{j}") for j in range(NB)]
    qg23_w = [ptile([128, 512], F16, f"qg23_{j}") for j in range(NB)]
    hid01_w = [ptile([128, 512], F16, f"hid01_{j}") for j in range(NB)]
    hid23_w = [ptile([128, 512], F16, f"hid23_{j}") for j in range(NB)]

    with ctx:
        in_pool = ctx.enter_context(tc.tile_pool(name="in_pool", bufs=5))
        qt_pool = ctx.enter_context(tc.tile_pool(name="qt_pool", bufs=2))
        vt_pool = ctx.enter_context(tc.tile_pool(name="vt_pool", bufs=2))
        pt_pool = ctx.enter_context(tc.tile_pool(name="pt_pool", bufs=3))
        rec_pool = ctx.enter_context(tc.tile_pool(name="rec_pool", bufs=2))
        stage_pool = ctx.enter_context(tc.tile_pool(name="stage", bufs=2))
        ps = ctx.enter_context(
            tc.tile_pool(name="ps", bufs=2, space="PSUM"))

        # quad tiles, filled by emit_dma, consumed by emit_proj
        quads = {t: [None] * 8 for t in "qkv"}

        def load_quad(t, idx, eng, src, P, qd):
            tl = in_pool.tile([128, 4, 1024], F16, tag=f"{t}quad",
                              name=f"{t}in_{idx}")
            quads[t][idx] = tl
            eng.dma_start(tl[:], src[:, qd, P])

        def emit_dma_head():
            """weights + pair-0 input quads + early consts (phase A)."""
            nc.sync.dma_start(wq_sb[:], wq[:])
            nc.sync.dma_start(wk_sb[:], wk[:])
            nc.sync.dma_start(wv_sb[:], wv[:])
            for qd in range(4):
                load_quad("q", qd, nc.sync, qT, 0, qd)
                load_quad("k", qd, nc.sync, kT, 0, qd)
                load_quad("v", qd, nc.sync, vT, 0, qd)
                yield
            for dst, srcap in ((bq_sb, bq), (bk_sb, bk), (bv_sb, bv),
                               (wg_sb, wg), (bg_sb, bg)):
                nc.sync.dma_start(dst[:], srcap[:])
            yield

        def emit_dma_tail():
            """pair-1 input quads + wfc (phase B, overlapped)."""
            for qd in range(4):
                load_quad("q", 4 + qd, nc.sync, qT, 1, qd)
                load_quad("k", 4 + qd, nc.sync, kT, 1, qd)
                load_quad("v", 4 + qd, nc.sync, vT, 1, qd)
                yield
            nc.sync.dma_start(wfc0_sb[:], wfc[0:128, :])
            yield
            nc.sync.dma_start(wfc1_sb[:], wfc[128:256, :])
            yield

        dma_gens = {}

        def ensure_quad(t, idx):
            gen = dma_gens[0] if idx < 4 else dma_gens[1]
            while quads[t][idx] is None:
                try:
                    next(gen)
                except StopIteration:
                    break
            assert quads[t][idx] is not None

        def emit_proj(P):
            """projections + G + V transpose for window pair P."""
            wins = (2 * P, 2 * P + 1)

            # Q/K psum pairs live in wide "st" tiles (attention is not
            # running during projections); V pair + G use the "mm" tag.
            q_ps = ps.tile([128, 1024], F32, tag="st", name="q_ps")
            k_ps = ps.tile([128, 1024], F32, tag="st", name="k_ps")
            v0_ps = ps.tile([64, 512], F32, tag="mm", name="v0_ps")
            v1_ps = ps.tile([64, 512], F32, tag="mm", name="v1_ps")
            for ec in range(16):
                qi = P * 4 + ec // 4
                for t, w_sb, dsts, mw in (
                        ("q", wq_sb, (q_ps[:, 0:512], q_ps[:, 512:1024]),
                         128),
                        ("k", wk_sb, (k_ps[:, 0:512], k_ps[:, 512:1024]),
                         128),
                        ("v", wv_sb, (v0_ps[:], v1_ps[:]), 64)):
                    ensure_quad(t, qi)
                    quad = quads[t][qi]
                    w = w_sb[:, bass.ts(ec, mw)]
                    nc.tensor.matmul(dsts[0], w, quad[:, ec % 4, 0:512],
                                     start=(ec == 0), stop=(ec == 15))
                    nc.tensor.matmul(dsts[1], w, quad[:, ec % 4, 512:1024],
                                     start=(ec == 0), stop=(ec == 15))
                    yield
            qt0 = qt_pool.tile([128, 512], F16, tag="qt", name="qt0")
            qt1 = qt_pool.tile([128, 512], F16, tag="qt", name="qt1")
            nc.scalar.activation(qt0[:], q_ps[:, 0:512], AF.Identity,
                                 bias=bq_sb[:])
            nc.scalar.activation(qt1[:], q_ps[:, 512:1024], AF.Identity,
                                 bias=bq_sb[:])
            nc.scalar.activation(kt_w[wins[0]][:], k_ps[:, 0:512],
                                 AF.Identity, bias=bk_sb[:])
            nc.scalar.activation(kt_w[wins[1]][:], k_ps[:, 512:1024],
                                 AF.Identity, bias=bk_sb[:])
            for wi, vsl in ((wins[0], v0_ps[:]),
                            (wins[1], v1_ps[:])):
                vt_sb = vt_pool.tile([64, 512], BF16, tag="vt", name="vt_sb")
                nc.scalar.activation(vt_sb[:], vsl, AF.Identity,
                                     bias=bv_sb[:])
                for c in range(4):
                    nc.sync.dma_start_transpose(
                        vo_w[wi][c][:, 0:64],
                        vt_sb[:, bass.ts(c, 128)])
            # --- G transform (row-tiled pair01 / pair23) ---
            for wi, qt in ((wins[0], qt0), (wins[1], qt1)):
                g01_ps = ps.tile([128, 512], F32, tag="mm", name="g01_ps")
                nc.tensor.matmul(g01_ps[:], wg_sb[0:64, 0:128], qt[0:64, :],
                                 start=True, stop=True)
                yield
                g23_ps = ps.tile([128, 512], F32, tag="mm", name="g23_ps")
                nc.tensor.matmul(g23_ps[:], wg_sb[64:128, 128:256],
                                 qt[64:128, :], start=True, stop=True)
                yield
                nc.scalar.activation(qg01_w[wi][:], g01_ps[:], AF.Identity,
                                     bias=bg_sb[:, 0:1])
                nc.scalar.activation(qg23_w[wi][:], g23_ps[:], AF.Identity,
                                     bias=bg_sb[:, 1:2])

        def emit_attn(j):
            klast = 4 * j + 3
            for p in range(2):  # g-pairs (2p, 2p+1)
                pv_a = ps.tile([65, 512], F32, tag="pv", name="pv_a")
                pv_b = ps.tile([65, 512], F32, tag="pv", name="pv_b")
                pending = None

                def flush(pend):
                    pt, k, off = pend
                    vsl = vo_w[k // 4][k % 4][:, 0:65]
                    nc.tensor.matmul(pv_a[:, off:512], vsl, pt[:, off:512],
                                     start=(k == 0), stop=(k == klast))
                    nc.tensor.matmul(pv_b[:, off:512], vsl,
                                     pt[:, 512 + off:1024],
                                     start=(k == 0), stop=(k == klast))

                qg = qg01_w[j] if p == 0 else qg23_w[j]
                for k in range(klast + 1):
                    kc = kt_w[k // 4][:, bass.ts(k % 4, 128)]
                    i = k - 4 * j
                    off = max(0, 128 * i)
                    st = ps.tile([128, 1024], F32, tag="st", name="st")
                    nc.tensor.matmul(st[:, off:512], kc[0:64, :],
                                     qg[0:64, off:512],
                                     start=True, stop=True)
                    nc.tensor.matmul(st[:, 512 + off:1024], kc[64:128, :],
                                     qg[64:128, off:512],
                                     start=True, stop=True)
                    yield
                    pt = pt_pool.tile([128, 1024], BF16, tag="pt", name="pt")
                    st3 = st[:].rearrange("p (h c) -> p h c", c=512)
                    pt3 = pt[:].rearrange("p (h c) -> p h c", c=512)
                    # exp(8S - 35): the shift cancels in the softmax ratio
                    # and keeps all exp outputs within fp16/bf16 range (HW
                    # ACT saturates 16-bit outputs near the fp16 max)
                    nc.scalar.activation(pt3[:, :, off:512],
                                         st3[:, :, off:512],
                                         AF.Exp, scale=8.0,
                                         bias=eshift_sb[:])
                    if i >= 0:
                        # zero out below-diagonal cols [off, off+128)
                        nc.vector.tensor_mul(pt3[:, :, off:off + 128],
                                             pt3[:, :, off:off + 128],
                                             mask3)
                    if pending is not None:
                        flush(pending)
                        yield
                    pending = (pt, k, off)
                flush(pending)
                yield
                # normalize: hid[half] = pv[0:64] * 1/pv[64]
                hid = hid01_w[j] if p == 0 else hid23_w[j]
                for half, pv in ((0, pv_a), (1, pv_b)):
                    # custom-DVE recip can't read PSUM on HW: stage to SBUF
                    den = rec_pool.tile([1, 512], F32, tag="den", name="den")
                    nc.vector.tensor_copy(den[:], pv[64:65, :])
                    rec = rec_pool.tile([1, 512], F32, tag="rec", name="rec")
                    nc.vector.reciprocal_approx_fast(rec[:], den[:])
                    recr = rec_pool.tile([64, 512], F32, tag="recr",
                                         name="recr")
                    nc.gpsimd.partition_broadcast(recr[:], rec[:])
                    if dumps is not None and j == 0 and p == 0 and half == 0:
                        nc.sync.dma_start(dumps["d_rec"][:], recr[:])
                    nc.vector.tensor_mul(hid[half * 64:half * 64 + 64, :],
                                         pv[0:64, :], recr[:])

        def emit_fc(j):
            for m in range(4):
                msl = bass.ts(m, 128)
                stage = stage_pool.tile([128, 2048], F16, tag="fco",
                                        name="stage")
                for eo in range(4):
                    fc_ps = ps.tile([128, 512], F32, tag="mm",
                                    name="fc_ps")
                    nc.tensor.matmul(fc_ps[:], hid01_w[j][:, msl],
                                     wfc0_sb[:, bass.ts(eo, 512)],
                                     start=True, stop=False)
                    yield
                    nc.tensor.matmul(fc_ps[:], hid23_w[j][:, msl],
                                     wfc1_sb[:, bass.ts(eo, 512)],
                                     start=False, stop=True)
                    yield
                    nc.vector.tensor_copy(stage[:, bass.ts(eo, 512)],
                                          fc_ps[:])
                nc.sync.dma_start(
                    out[512 * j + 128 * m: 512 * j + 128 * m + 128, :],
                    stage[:])

        from itertools import chain as ichain

        def drain(g):
            for _ in g:
                pass

        def rr(pairs):
            """round-robin emission: [(generator, steps_per_turn)]"""
            live = [[g, w] for g, w in pairs]
            while live:
                for gw in list(live):
                    g, w = gw
                    try:
                        for _ in range(w):
                            next(g)
                    except StopIteration:
                        live.remove(gw)

        # Phase A: DMA ramp + pair-0 projections
        dma_gens[0] = emit_dma_head()
        dma_gens[1] = emit_dma_tail()
        rr([(dma_gens[0], 2), (emit_proj(0), 3)])
        # Phase B: window-0/1 attention + pair-1 projections + tail DMAs
        rr([(ichain(emit_attn(0), emit_attn(1)), 1),
            (emit_proj(1), 2),
            (dma_gens[1], 1)])
        # Later: attention with FC of completed windows as PE filler
        rr([(emit_attn(2), 3), (emit_fc(0), 2)])
        rr([(emit_attn(3), 1), (ichain(emit_fc(1), emit_fc(2)), 1)])
        drain(emit_fc(3))

        if dumps is not None:
            nc.sync.dma_start(dumps["d_kt0"][:], kt_w[0][0:64, :])
            nc.sync.dma_start(dumps["d_qg00"][:], qg01_w[0][0:64, :])
            for c in range(4):
                nc.sync.dma_start(dumps["d_vo0"][:, c, :], vo_w[0][c][:, :])
            nc.sync.dma_start(dumps["d_hid01_0"][:], hid01_w[0][:, :])


def shard_inputs(inputs):
    """full inputs -> list of 8 per-core in_maps (numpy, device layouts)"""
    f16 = np.float16
    f32 = np.float32
    q = np.asarray(inputs["q"], f32)[0]
    k = np.asarray(inputs["k"], f32)[0]
    v = np.asarray(inputs["v"], f32)[0]
    Wq = np.asarray(inputs["Wq"], f32)
    Wk = np.asarray(inputs["Wk"], f32)
    Wv = np.asarray(inputs["Wv"], f32)
    bq = np.asarray(inputs["bq"], f32)
    bk = np.asarray(inputs["bk"], f32)
    bv = np.asarray(inputs["bv"], f32)
    WG = np.asarray(inputs["WG"], f32)
    bG = np.asarray(inputs["bG"], f32)
    Wfc = np.asarray(inputs["Wfc"], f32)

    def swizzle(x):
        # x [N, E] -> xT [E, N] -> [p, qd, P, e, c]
        xt = x.T.astype(f16).reshape(4, 4, 128, 2, 1024)   # (qd, e, p, P, c)
        return np.ascontiguousarray(xt.transpose(2, 0, 3, 1, 4))

    qT = swizzle(q)
    kT = swizzle(k)
    vT = swizzle(v)

    def chunked(w):
        # [E, 64] -> [128, 16*64]: e-chunk ec at cols [64ec, 64ec+64)
        M = w.shape[1]
        return np.ascontiguousarray(
            w.reshape(16, 128, M).transpose(1, 0, 2).reshape(128, 16 * M))

    maps = []
    for h in range(HK):
        sl = slice(h * D, (h + 1) * D)
        wq_h = Wq[:, sl]
        wk_h = Wk[:, sl]
        m = {
            "qT": qT, "kT": kT, "vT": vT,
            "wq": chunked(np.concatenate([wq_h, wq_h], 1)).astype(f16),
            "wk": chunked(np.concatenate([wk_h, wk_h], 1)).astype(f16),
            "wv": chunked(Wv[:, sl]).astype(f16),
            "bq": np.concatenate([bq[sl], bq[sl]]).reshape(128, 1).copy(),
            "bk": np.concatenate([bk[sl], bk[sl]]).reshape(128, 1).copy(),
            "bv": bv[sl].reshape(64, 1).copy(),
            "wg": np.concatenate([WG[h], WG[h]], 0).astype(f16),  # [128, 256]
            "bg": np.ascontiguousarray(
                bG[h].reshape(2, 128).T).astype(f32),     # [128, 2]
            "wfc": Wfc[h * 256:(h + 1) * 256, :].astype(f16),
        }
        maps.append(m)
    return maps


_compiled = None
last_results = None


def get_compiled():
    global _compiled
    if _compiled is None:
        _compiled = build_program()
    return _compiled


def kernel(**inputs):
    global last_results
    nc = get_compiled()
    in_maps = shard_inputs(inputs)
    last_results = bass_utils.run_bass_kernel_spmd(
        nc, in_maps, core_ids=list(range(8)))
    bfc = np.asarray(inputs["bfc"], np.float32)
    acc = np.zeros((N, E), np.float64)
    for res in last_results.results:
        acc += res["out"].astype(np.float64)
    full = (acc + bfc[None, :].astype(np.float64)).astype(np.float32)
    return full.reshape(1, N, E)

